# revision 36
# baseline (speedup 1.0000x reference)
"""Expected Calibration Error (ECE) kernel for Trainium2, 8 NeuronCores.

Problem: inputs [2e6, 128] f32 row-probabilities, targets [2e6] int64/int32.
  conf_i = max_c inputs[i, c];  pred_i = argmax_c inputs[i, c]
  bin_i  = bucketize(conf_i, linspace(0, 1, 11), right=True) - 1
  ECE    = sum_b |corr_sum[b] - conf_sum[b]| / N

Strategy (data-parallel over rows, 250k rows per core):
  Host packs each probability into a uint16 sort key
      key16[i, c] = round(v * Kscale) * 128 + (127 - c),   Kscale = 511 / vmax
  (9-bit quantized value, 7-bit first-index tie-break).  The key is monotone
  in v and ties between equal quantized values break toward the smaller
  class, so max_c key16 yields both the quantized confidence and a
  deterministic winner class in one associative max:
      K = max_c key16;  q = K >> 7;  c_w = 127 - (K & 127)
      correct = (c_w == target);  conf ~= q / Kscale
  (host-validated: rel err ~2e-3 vs the f32 reference; winner differs from
  f32 argmax only on quantization ties, a zero-mean ~0.5% fluctuation).

  Device per supertile [128 part, 64 rows/part, 128 classes] (u16, 2 MB):
    4 x tensor_tensor(max) tree levels     (DVE 2x perf mode for 16-bit)
    1 x tensor_reduce(max, axis=X) on 8    (DVE 1x)            -> K[128, 64]
  (measured: TT u16 runs 2 elem/cycle, TENSOR_REDUCE only 1 -> tree+tail is
  ~1.65x faster than one big reduce)
  Per chunk of K columns (overlapped with the stream, DVE + ScalarE):
    kf = f32(K) - 63.5; r = magic2^30(kf) = q*128; correct = (kf-r == 63.5-t)
    G_b = sign(r - E_b) in {-1,+1} cumulative bin masks  (ScalarE, idle)
    diag-batched PE matmul accumulates [2*16, 10*16] PSUM
  Host: extract+sum diagonal [2,10] blocks over cores, sign-fixup, per-bin
  differences, |.| sum, / N.

Sharding: rows split evenly, 250,000 per core = 30 supertiles x 8192 rows
(contiguous 2 MB DMA, 16 KB per partition; supertile 0 quartered for a fast
ramp) + one [128, 33, 128] tail supertile + one [16, 1, 128] mini-tail.
Input DMA alternates between the SP and Activation hardware DGE queues
(one queue saturates ~300 GB/s; two reach the ~372 GB/s per-core HBM share).
"""

import numpy as np

N = 2_000_000
C = 128
NCORES = 8
ROWS = N // NCORES            # 250_000
S = 96                        # rows per partition per supertile
ST_ROWS = 128 * S             # 12288
NST = ROWS // ST_ROWS         # 20 supertiles -> 245_760 rows
TAIL_S = 33                   # tail supertile [128, 33, 128] -> 4224 rows
TAIL2_P = 16                  # mini-tail [16, 1, 128] -> 16 rows
NTG = NST * S + TAIL_S + 1    # 1954 key columns per core

CHUNK_SIZES = [288] * 6 + [96, 96, TAIL_S + 1]
assert sum(CHUNK_SIZES) == NTG
CHUNK_STARTS = [sum(CHUNK_SIZES[:i]) for i in range(len(CHUNK_SIZES))]
NCHUNKS = len(CHUNK_SIZES)
MAXCH = max(CHUNK_SIZES)
MMB = 16                      # matmul diagonal-batch block (columns)

QMAX = 511
MAGIC = float(2 ** 30)        # f32 ulp at 2^30 is 128 -> rounds to mult of 128

_EDGES_F32 = np.linspace(0.0, 1.0, 11).astype(np.float32)  # matches jnp.linspace


def _kscale(vmax: float) -> float:
    return QMAX / float(vmax)


def _edges_scaled(kscale: float) -> list[float]:
    # bin b threshold: row in bin >= b  <=>  q >= ceil(edge_b * kscale);
    # sign threshold strictly between multiples of 128 so Sign never sees 0
    out = []
    for b in range(1, 10):
        qb = np.ceil(float(_EDGES_F32[b]) * kscale)
        out.append((qb - 0.5) * 128.0)
    return out


_NC_CACHE: dict = {}


def _build_bass(kscale: float):
    key = round(kscale, 6)
    if key in _NC_CACHE:
        return _NC_CACHE[key]

    import concourse.bacc as bacc
    import concourse.tile as tile
    from concourse import mybir

    edges = _edges_scaled(kscale)  # E_b for b = 1..9

    nc = bacc.Bacc()
    f32 = mybir.dt.float32
    u16 = mybir.dt.uint16
    mx = mybir.AluOpType.max
    x = nc.dram_tensor("x", [ROWS, C], u16, kind="ExternalInput")
    tg = nc.dram_tensor("tg", [128, NTG], f32, kind="ExternalInput")
    out = nc.dram_tensor("out", [2 * MMB, 10 * MMB], f32, kind="ExternalOutput")

    with tile.TileContext(nc) as tc:
        with (
            tc.tile_pool(name="persist", bufs=1) as persist,
            tc.tile_pool(name="inbuf", bufs=4) as inbuf,
            tc.tile_pool(name="qbuf", bufs=1) as qbuf,
            tc.tile_pool(name="treebuf", bufs=1) as treebuf,
            tc.tile_pool(name="tailbuf", bufs=1) as tailbuf,
            tc.tile_pool(name="decbuf", bufs=2) as decbuf,
            tc.tile_pool(name="psum", bufs=1, space="PSUM") as psumpool,
        ):
            x_ap = x[:]
            xr0 = x_ap[: NST * ST_ROWS, :].rearrange(
                "(s p k) c -> s p (k c)", s=NST, p=128, k=S
            )
            # supertile 0 split into quarters (free-dim slices of the same
            # row layout) so DVE compute starts right after the first 512 KB
            QS = 24
            qtiles = []
            for qi in range(4):
                t = qbuf.tile([128, QS, C], u16, name=f"q{qi}", tag=f"q{qi}")
                eng = nc.sync if qi % 2 == 0 else nc.scalar
                eng.dma_start(
                    out=t[:],
                    in_=xr0[0][:, qi * QS * C : (qi + 1) * QS * C],
                )
                qtiles.append(t)

            tg_tiles = [
                persist.tile(
                    [128, CHUNK_SIZES[c]], f32, name=f"tgt{c}", tag=f"tgt{c}"
                )
                for c in range(NCHUNKS)
            ]
            edge_bias = persist.tile([128, 10], f32)
            for b in range(1, 10):
                nc.vector.memset(edge_bias[:, b : b + 1], -edges[b - 1])
            for c in range(NCHUNKS):
                a = CHUNK_STARTS[c]
                nc.gpsimd.dma_start(
                    out=tg_tiles[c][:], in_=tg[:][:, a : a + CHUNK_SIZES[c]]
                )

            kc_tiles = [
                persist.tile(
                    [128, CHUNK_SIZES[c]], u16, name=f"kc{c}", tag=f"kc{c}"
                )
                for c in range(NCHUNKS)
            ]
            # mini-tail column: partitions >= TAIL2_P are never written
            nc.vector.memset(kc_tiles[-1][:], 0.0)

            psum = psumpool.tile([2 * MMB, 10 * MMB], f32)
            xr = xr0

            mm_state = {"first": True}
            total_mms = sum((csz + MMB - 1) // MMB for csz in CHUNK_SIZES)
            mm_done = [0]

            def emit_chunk_epilogue(c, on_vector=False):
                csz = CHUNK_SIZES[c]
                kt = kc_tiles[c]
                # walrus supports Pool TensorScalar but not TensorTensor:
                # cast+magic ride gpsimd, the rest stays on Vector
                eng = nc.vector if on_vector else nc.gpsimd
                kf = decbuf.tile([128, MAXCH], f32, name="kf", tag="kf")
                cc = decbuf.tile([128, MAXCH, 2], f32, name="cc", tag="cc")
                g = decbuf.tile([128, MAXCH, 10], f32, name="g", tag="g")
                t2 = decbuf.tile([128, MAXCH], f32, name="t2", tag="t2")
                # kf = float(K) - 63.5 (cast folded in; the .5 keeps the magic
                # rounding tie-free)
                eng.tensor_scalar(
                    out=kf[:, :csz],
                    in0=kt[:, :csz],
                    scalar1=63.5,
                    scalar2=None,
                    op0=mybir.AluOpType.subtract,
                )
                # r = nearest-mult-of-128(kf) = q*128 (f32 ulp at 2^30 is 128)
                eng.tensor_scalar(
                    out=cc[:, :csz, 0],
                    in0=kf[:, :csz],
                    scalar1=MAGIC,
                    scalar2=MAGIC,
                    op0=mybir.AluOpType.add,
                    op1=mybir.AluOpType.subtract,
                )
                # cdiff = kf - r = 63.5 - c_w;  correct = (cdiff == 63.5 - t)
                nc.vector.tensor_tensor(
                    out=t2[:, :csz],
                    in0=kf[:, :csz],
                    in1=cc[:, :csz, 0],
                    op=mybir.AluOpType.subtract,
                )
                nc.vector.tensor_tensor(
                    out=cc[:, :csz, 1],
                    in0=t2[:, :csz],
                    in1=tg_tiles[c][:, :csz],
                    op=mybir.AluOpType.is_equal,
                )
                # G_0 = 1; G_b = sign(r - E_b) in {-1, +1} for b in 1..9
                if on_vector:
                    nc.vector.memset(g[:, :csz, 0], 1.0)
                    for b in range(1, 10):
                        nc.vector.tensor_scalar(
                            out=g[:, :csz, b],
                            in0=cc[:, :csz, 0],
                            scalar1=edge_bias[:, b : b + 1],
                            scalar2=2.0,
                            op0=mybir.AluOpType.is_ge,
                            op1=mybir.AluOpType.mult,
                        )
                    # {0,2} -> {-1,+1} in one strided pass over planes 1..9
                    nc.vector.tensor_scalar(
                        out=g[:, :csz, 1:10],
                        in0=g[:, :csz, 1:10],
                        scalar1=1.0,
                        scalar2=None,
                        op0=mybir.AluOpType.subtract,
                    )
                else:
                    nc.scalar.activation(
                        out=g[:, :csz, 0],
                        in_=kf[:, :csz],
                        func=mybir.ActivationFunctionType.Copy,
                        bias=1.0,
                        scale=0.0,
                    )
                    for b in range(1, 10):
                        nc.scalar.activation(
                            out=g[:, :csz, b],
                            in_=cc[:, :csz, 0],
                            func=mybir.ActivationFunctionType.Sign,
                            bias=edge_bias[:, b : b + 1],
                            scale=1.0,
                        )
                nmm = (csz + MMB - 1) // MMB
                if csz % MMB:
                    pad = nmm * MMB
                    nc.vector.memset(cc[:, csz:pad, :], 0.0)
                    nc.vector.memset(g[:, csz:pad, :], 0.0)
                for blk in range(nmm):
                    lo = blk * MMB
                    hi = lo + MMB
                    mm_done[0] += 1
                    nc.tensor.matmul(
                        psum[:],
                        lhsT=cc[:, lo:hi, :].rearrange("p a b -> p (a b)"),
                        rhs=g[:, lo:hi, :].rearrange("p a b -> p (a b)"),
                        start=mm_state["first"],
                        stop=mm_done[0] == total_mms,
                    )
                    mm_state["first"] = False

            def _tree(t, npart, nseg):
                h = treebuf.tile([128, S, 64], u16, name="h", tag="h")
                q = treebuf.tile([128, S, 32], u16, name="q", tag="q")
                r3 = treebuf.tile([128, S, 16], u16, name="r3", tag="r3")
                r4 = treebuf.tile([128, S, 8], u16, name="r4", tag="r4")
                nc.vector.tensor_tensor(
                    out=h[:npart, :nseg, :],
                    in0=t[:npart, :nseg, :64],
                    in1=t[:npart, :nseg, 64:],
                    op=mx,
                )
                nc.vector.tensor_tensor(
                    out=q[:npart, :nseg, :],
                    in0=h[:npart, :nseg, :32],
                    in1=h[:npart, :nseg, 32:],
                    op=mx,
                )
                nc.vector.tensor_tensor(
                    out=r3[:npart, :nseg, :],
                    in0=q[:npart, :nseg, :16],
                    in1=q[:npart, :nseg, 16:],
                    op=mx,
                )
                nc.vector.tensor_tensor(
                    out=r4[:npart, :nseg, :],
                    in0=r3[:npart, :nseg, :8],
                    in1=r3[:npart, :nseg, 8:],
                    op=mx,
                )
                return r4

            def tree_reduce(t, npart, nseg, kc_ap):
                """[npart, nseg, 128] u16 -> max over classes -> kc_ap [npart, nseg]."""
                r4 = _tree(t, npart, nseg)
                nc.vector.tensor_reduce(
                    out=kc_ap,
                    in_=r4[:npart, :nseg, :],
                    axis=mybir.AxisListType.X,
                    op=mx,
                )

            def tree_reduce2(t, kc_a, kc_b, w):
                """Full-S supertile whose columns straddle a chunk boundary."""
                r4 = _tree(t, 128, S)
                nc.vector.tensor_reduce(
                    out=kc_a,
                    in_=r4[:, :w, :],
                    axis=mybir.AxisListType.X,
                    op=mx,
                )
                nc.vector.tensor_reduce(
                    out=kc_b,
                    in_=r4[:, w:, :],
                    axis=mybir.AxisListType.X,
                    op=mx,
                )

            st_tiles = {}

            def load_st(si):
                t = inbuf.tile([128, S, C], u16, name="xt", tag="xt")
                eng = nc.sync if si % 2 == 0 else nc.scalar
                eng.dma_start(out=t[:], in_=xr[si])
                st_tiles[si] = t

            for si in range(1, min(5, NST)):
                load_st(si)

            # tail supertile rows 245760..249984 and mini-tail 249984..250000
            xt_tail = tailbuf.tile([128, TAIL_S, C], u16)
            nc.gpsimd.dma_start(
                out=xt_tail[:],
                in_=x_ap[NST * ST_ROWS : NST * ST_ROWS + 128 * TAIL_S, :]
                .rearrange("(p k) c -> p (k c)", p=128, k=TAIL_S),
            )
            xt_tail2 = tailbuf.tile([TAIL2_P, 1, C], u16)
            nc.gpsimd.dma_start(
                out=xt_tail2[:],
                in_=x_ap[NST * ST_ROWS + 128 * TAIL_S :, :].rearrange(
                    "(p k) c -> p (k c)", p=TAIL2_P, k=1
                ),
            )

            def chunk_of(j):
                c = 0
                while c + 1 < NCHUNKS and CHUNK_STARTS[c + 1] <= j:
                    c += 1
                return c

            fired = [0]
            for si in range(NST):
                if si == 0:
                    # supertile 0 from its 4 quarter tiles
                    for qi in range(4):
                        j = qi * QS
                        tree_reduce(
                            qtiles[qi], 128, QS, kc_tiles[0][:, j : j + QS]
                        )
                else:
                    t = st_tiles.pop(si)
                    if si + 4 < NST:
                        load_st(si + 4)
                    j = si * S
                    c = chunk_of(j)
                    l = j - CHUNK_STARTS[c]
                    if l + S <= CHUNK_SIZES[c]:
                        tree_reduce(t, 128, S, kc_tiles[c][:, l : l + S])
                    else:
                        # supertile straddles a chunk boundary: two TR halves
                        w = CHUNK_SIZES[c] - l
                        tree_reduce2(
                            t,
                            kc_tiles[c][:, l : l + w],
                            kc_tiles[c + 1][:, : S - w],
                            w,
                        )
                if si == 4:
                    # tail reduces mid-stream, away from busy start/end
                    tree_reduce(
                        xt_tail, 128, TAIL_S, kc_tiles[-1][:, :TAIL_S]
                    )
                    nc.vector.tensor_reduce(
                        out=kc_tiles[-1][:TAIL2_P, TAIL_S : TAIL_S + 1],
                        in_=xt_tail2[:],
                        axis=mybir.AxisListType.X,
                        op=mx,
                    )
                done_cols = (si + 1) * S
                while (
                    fired[0] < NCHUNKS - 2
                    and CHUNK_STARTS[fired[0]] + CHUNK_SIZES[fired[0]]
                    <= done_cols
                ):
                    emit_chunk_epilogue(fired[0])
                    fired[0] += 1
                if si == 6 and fired[0] >= 1:
                    # tail chunk fires early (its reduces ran at si == 4)
                    emit_chunk_epilogue(NCHUNKS - 1)

            while fired[0] < NCHUNKS - 1:
                emit_chunk_epilogue(fired[0], on_vector=fired[0] == NCHUNKS - 2)
                fired[0] += 1

            res = persist.tile([2 * MMB, 10 * MMB], f32)
            nc.vector.tensor_copy(out=res[:], in_=psum[:])
            nc.sync.dma_start(out=out[:], in_=res[:])

    nc.finalize()
    _NC_CACHE[key] = nc
    return nc


def _pack_keys(x_loc: np.ndarray, kscale: float) -> np.ndarray:
    """[ROWS, C] f32 -> uint16 keys q*128 + (127 - c)."""
    q = np.rint(x_loc * np.float32(kscale)).astype(np.uint16)
    inv_c = (127 - np.arange(C, dtype=np.uint16)).astype(np.uint16)
    return ((q << 7) | inv_c[None, :]).astype(np.uint16)


def _prep_targets(t_loc: np.ndarray) -> np.ndarray:
    """[ROWS] int targets -> [128, NTG] f32 holding 63.5 - target per column."""
    s0 = np.float32(63.5) - t_loc.astype(np.float32)
    tgc = np.full((128, NTG), -1000.5, dtype=np.float32)
    main = s0[: NST * ST_ROWS].reshape(NST, 128, S)
    tgc[:, : NST * S] = main.transpose(1, 0, 2).reshape(128, NST * S)
    tail = s0[NST * ST_ROWS : NST * ST_ROWS + 128 * TAIL_S].reshape(128, TAIL_S)
    tgc[:, NST * S : NST * S + TAIL_S] = tail
    tgc[:TAIL2_P, NTG - 1] = s0[NST * ST_ROWS + 128 * TAIL_S :]
    return tgc


def _run(inputs: np.ndarray, targets: np.ndarray, trace: bool = False):
    from concourse.bass_utils import run_bass_kernel_spmd

    inputs = np.ascontiguousarray(inputs, dtype=np.float32)
    targets = np.asarray(targets)
    vmax = float(inputs.max())
    kscale = _kscale(vmax)

    nc = _build_bass(kscale)

    in_maps = []
    for k in range(NCORES):
        lo = k * ROWS
        xk = _pack_keys(inputs[lo : lo + ROWS], kscale)
        tgc = _prep_targets(targets[lo : lo + ROWS])
        in_maps.append({"x": xk, "tg": tgc})

    last_err = None
    for _attempt in range(3):
        try:
            r = run_bass_kernel_spmd(
                nc, in_maps, core_ids=list(range(NCORES)), trace=trace
            )
            break
        except Exception as e:  # transient NRT_EXEC_UNIT_UNRECOVERABLE on cold device
            last_err = e
    else:
        raise last_err
    return r, kscale


def _combine(results, kscale) -> np.ndarray:
    Ssign = np.zeros((2, 10), dtype=np.float64)
    for r in results:
        o = r["out"].astype(np.float64).reshape(MMB, 2, MMB, 10)
        Ssign += np.einsum("aman->mn", o)
    # G_b in {-1,+1} for b>=1, G_0 = 1: [x >= E_b] = (S_b + S_0) / 2
    Ssign[:, 1:] = (Ssign[:, 1:] + Ssign[:, 0:1]) / 2.0
    Sq = Ssign[0] / (128.0 * kscale)   # cumulative conf sums (conf units)
    Sk = Ssign[1]                      # cumulative correct counts
    conf_sum = Sq - np.append(Sq[1:], 0.0)
    corr_sum = Sk - np.append(Sk[1:], 0.0)
    ece = np.abs(corr_sum - conf_sum).sum() / N
    return np.asarray(ece, dtype=np.float32)


def kernel(inputs: np.ndarray, targets: np.ndarray) -> np.ndarray:
    r, kscale = _run(inputs, targets, trace=False)
    return _combine(r.results, kscale)


# revision 37
# speedup vs baseline: 1.5845x; 1.5845x over previous
"""Expected Calibration Error (ECE) kernel for Trainium2, 8 NeuronCores.

Problem: inputs [2e6, 128] f32 row-probabilities, targets [2e6] int64/int32.
  conf_i = max_c inputs[i, c];  pred_i = argmax_c inputs[i, c]
  bin_i  = bucketize(conf_i, linspace(0, 1, 11), right=True) - 1
  ECE    = sum_b |corr_sum[b] - conf_sum[b]| / N

Strategy (data-parallel over rows, 250k rows per core):
  Host packs each probability into a uint16 sort key
      key16[i, c] = round(v * Kscale) * 128 + (127 - c),   Kscale = 511 / vmax
  (9-bit quantized value, 7-bit first-index tie-break).  The key is monotone
  in v and ties between equal quantized values break toward the smaller
  class, so max_c key16 yields both the quantized confidence and a
  deterministic winner class in one associative max:
      K = max_c key16;  q = K >> 7;  c_w = 127 - (K & 127)
      correct = (c_w == target);  conf ~= q / Kscale
  (host-validated: rel err ~2e-3 vs the f32 reference; winner differs from
  f32 argmax only on quantization ties, a zero-mean ~0.5% fluctuation).

  Device per supertile [128 part, 64 rows/part, 128 classes] (u16, 2 MB):
    4 x tensor_tensor(max) tree levels     (DVE 2x perf mode for 16-bit)
    1 x tensor_reduce(max, axis=X) on 8    (DVE 1x)            -> K[128, 64]
  (measured: TT u16 runs 2 elem/cycle, TENSOR_REDUCE only 1 -> tree+tail is
  ~1.65x faster than one big reduce)
  Per chunk of K columns (overlapped with the stream, DVE + ScalarE):
    kf = f32(K) - 63.5; r = magic2^30(kf) = q*128; correct = (kf-r == 63.5-t)
    G_b = sign(r - E_b) in {-1,+1} cumulative bin masks  (ScalarE, idle)
    diag-batched PE matmul accumulates [2*16, 10*16] PSUM
  Host: extract+sum diagonal [2,10] blocks over cores, sign-fixup, per-bin
  differences, |.| sum, / N.

Sharding: rows split evenly, 250,000 per core = 30 supertiles x 8192 rows
(contiguous 2 MB DMA, 16 KB per partition; supertile 0 quartered for a fast
ramp) + one [128, 33, 128] tail supertile + one [16, 1, 128] mini-tail.
Input DMA alternates between the SP and Activation hardware DGE queues
(one queue saturates ~300 GB/s; two reach the ~372 GB/s per-core HBM share).
"""

import numpy as np

N = 2_000_000
C = 128
NCORES = 8
ROWS = N // NCORES            # 250_000
S = 64                        # rows per partition per supertile
ST_ROWS = 128 * S             # 8192
NST = ROWS // ST_ROWS         # 30 supertiles -> 245_760 rows
TAIL_S = 33                   # tail supertile [128, 33, 128] -> 4224 rows
TAIL2_P = 16                  # mini-tail [16, 1, 128] -> 16 rows
NTG = NST * S + TAIL_S + 1    # 1954 key columns per core

CHUNK_SIZES = [256] * 6 + [256, 96, 32, TAIL_S + 1]
assert sum(CHUNK_SIZES) == NTG
CHUNK_STARTS = [sum(CHUNK_SIZES[:i]) for i in range(len(CHUNK_SIZES))]
NCHUNKS = len(CHUNK_SIZES)
MAXCH = max(CHUNK_SIZES)
MMB = 16                      # matmul diagonal-batch block (columns)

QMAX = 511
MAGIC = float(2 ** 30)        # f32 ulp at 2^30 is 128 -> rounds to mult of 128

_EDGES_F32 = np.linspace(0.0, 1.0, 11).astype(np.float32)  # matches jnp.linspace


def _kscale(vmax: float) -> float:
    return QMAX / float(vmax)


def _edges_scaled(kscale: float) -> list[float]:
    # bin b threshold: row in bin >= b  <=>  q >= ceil(edge_b * kscale);
    # sign threshold strictly between multiples of 128 so Sign never sees 0
    out = []
    for b in range(1, 10):
        qb = np.ceil(float(_EDGES_F32[b]) * kscale)
        out.append((qb - 0.5) * 128.0)
    return out


_NC_CACHE: dict = {}


def _build_bass(kscale: float):
    key = round(kscale, 6)
    if key in _NC_CACHE:
        return _NC_CACHE[key]

    import concourse.bacc as bacc
    import concourse.tile as tile
    from concourse import mybir

    edges = _edges_scaled(kscale)  # E_b for b = 1..9

    nc = bacc.Bacc()
    f32 = mybir.dt.float32
    u16 = mybir.dt.uint16
    mx = mybir.AluOpType.max
    x = nc.dram_tensor("x", [ROWS, C], u16, kind="ExternalInput")
    tg = nc.dram_tensor("tg", [128, NTG], f32, kind="ExternalInput")
    out = nc.dram_tensor("out", [2 * MMB, 10 * MMB], f32, kind="ExternalOutput")

    with tile.TileContext(nc) as tc:
        with (
            tc.tile_pool(name="persist", bufs=1) as persist,
            tc.tile_pool(name="inbuf", bufs=5) as inbuf,
            tc.tile_pool(name="qbuf", bufs=1) as qbuf,
            tc.tile_pool(name="treebuf", bufs=2) as treebuf,
            tc.tile_pool(name="tailbuf", bufs=1) as tailbuf,
            tc.tile_pool(name="decbuf", bufs=2) as decbuf,
            tc.tile_pool(name="psum", bufs=1, space="PSUM") as psumpool,
        ):
            x_ap = x[:]
            xr0 = x_ap[: NST * ST_ROWS, :].rearrange(
                "(s p k) c -> s p (k c)", s=NST, p=128, k=S
            )
            # supertile 0 split into quarters (free-dim slices of the same
            # row layout) so DVE compute starts right after the first 512 KB
            QS = 16
            qtiles = []
            for qi in range(4):
                t = qbuf.tile([128, QS, C], u16, name=f"q{qi}", tag=f"q{qi}")
                eng = nc.sync if qi % 2 == 0 else nc.scalar
                eng.dma_start(
                    out=t[:],
                    in_=xr0[0][:, qi * QS * C : (qi + 1) * QS * C],
                )
                qtiles.append(t)

            tg_tiles = [
                persist.tile(
                    [128, CHUNK_SIZES[c]], f32, name=f"tgt{c}", tag=f"tgt{c}"
                )
                for c in range(NCHUNKS)
            ]
            edge_bias = persist.tile([128, 10], f32)
            for b in range(1, 10):
                nc.vector.memset(edge_bias[:, b : b + 1], -edges[b - 1])
            for c in range(NCHUNKS):
                a = CHUNK_STARTS[c]
                nc.gpsimd.dma_start(
                    out=tg_tiles[c][:], in_=tg[:][:, a : a + CHUNK_SIZES[c]]
                )

            kc_tiles = [
                persist.tile(
                    [128, CHUNK_SIZES[c]], u16, name=f"kc{c}", tag=f"kc{c}"
                )
                for c in range(NCHUNKS)
            ]
            # mini-tail column: partitions >= TAIL2_P are never written
            nc.vector.memset(kc_tiles[-1][:], 0.0)

            psum = psumpool.tile([2 * MMB, 10 * MMB], f32)
            xr = xr0

            mm_state = {"first": True}
            total_mms = sum((csz + MMB - 1) // MMB for csz in CHUNK_SIZES)
            mm_done = [0]

            def emit_chunk_epilogue(c, on_vector=False):
                csz = CHUNK_SIZES[c]
                kt = kc_tiles[c]
                # walrus has no Pool-engine codegen for TensorTensor; decode
                # arithmetic stays on the Vector engine
                eng = nc.vector
                kf = decbuf.tile([128, MAXCH], f32, name="kf", tag="kf")
                cc = decbuf.tile([128, MAXCH, 2], f32, name="cc", tag="cc")
                g = decbuf.tile([128, MAXCH, 10], f32, name="g", tag="g")
                t2 = decbuf.tile([128, MAXCH], f32, name="t2", tag="t2")
                # kf = float(K) - 63.5 (cast folded in; the .5 keeps the magic
                # rounding tie-free)
                eng.tensor_scalar(
                    out=kf[:, :csz],
                    in0=kt[:, :csz],
                    scalar1=63.5,
                    scalar2=None,
                    op0=mybir.AluOpType.subtract,
                )
                # r = nearest-mult-of-128(kf) = q*128 (f32 ulp at 2^30 is 128)
                eng.tensor_scalar(
                    out=cc[:, :csz, 0],
                    in0=kf[:, :csz],
                    scalar1=MAGIC,
                    scalar2=MAGIC,
                    op0=mybir.AluOpType.add,
                    op1=mybir.AluOpType.subtract,
                )
                # cdiff = kf - r = 63.5 - c_w;  correct = (cdiff == 63.5 - t)
                eng.tensor_tensor(
                    out=t2[:, :csz],
                    in0=kf[:, :csz],
                    in1=cc[:, :csz, 0],
                    op=mybir.AluOpType.subtract,
                )
                eng.tensor_tensor(
                    out=cc[:, :csz, 1],
                    in0=t2[:, :csz],
                    in1=tg_tiles[c][:, :csz],
                    op=mybir.AluOpType.is_equal,
                )
                # G_0 = 1; G_b = sign(r - E_b) in {-1, +1} for b in 1..9
                if on_vector:
                    nc.vector.memset(g[:, :csz, 0], 1.0)
                    for b in range(1, 10):
                        nc.vector.tensor_scalar(
                            out=g[:, :csz, b],
                            in0=cc[:, :csz, 0],
                            scalar1=edge_bias[:, b : b + 1],
                            scalar2=2.0,
                            op0=mybir.AluOpType.is_ge,
                            op1=mybir.AluOpType.mult,
                        )
                    # {0,2} -> {-1,+1} in one strided pass over planes 1..9
                    nc.vector.tensor_scalar(
                        out=g[:, :csz, 1:10],
                        in0=g[:, :csz, 1:10],
                        scalar1=1.0,
                        scalar2=None,
                        op0=mybir.AluOpType.subtract,
                    )
                else:
                    nc.scalar.activation(
                        out=g[:, :csz, 0],
                        in_=kf[:, :csz],
                        func=mybir.ActivationFunctionType.Copy,
                        bias=1.0,
                        scale=0.0,
                    )
                    for b in range(1, 10):
                        nc.scalar.activation(
                            out=g[:, :csz, b],
                            in_=cc[:, :csz, 0],
                            func=mybir.ActivationFunctionType.Sign,
                            bias=edge_bias[:, b : b + 1],
                            scale=1.0,
                        )
                nmm = (csz + MMB - 1) // MMB
                if csz % MMB:
                    pad = nmm * MMB
                    nc.vector.memset(cc[:, csz:pad, :], 0.0)
                    nc.vector.memset(g[:, csz:pad, :], 0.0)
                for blk in range(nmm):
                    lo = blk * MMB
                    hi = lo + MMB
                    mm_done[0] += 1
                    nc.tensor.matmul(
                        psum[:],
                        lhsT=cc[:, lo:hi, :].rearrange("p a b -> p (a b)"),
                        rhs=g[:, lo:hi, :].rearrange("p a b -> p (a b)"),
                        start=mm_state["first"],
                        stop=mm_done[0] == total_mms,
                    )
                    mm_state["first"] = False

            def _tree(t, npart, nseg):
                h = treebuf.tile([128, S, 64], u16, name="h", tag="h")
                q = treebuf.tile([128, S, 32], u16, name="q", tag="q")
                r3 = treebuf.tile([128, S, 16], u16, name="r3", tag="r3")
                r4 = treebuf.tile([128, S, 8], u16, name="r4", tag="r4")
                nc.vector.tensor_tensor(
                    out=h[:npart, :nseg, :],
                    in0=t[:npart, :nseg, :64],
                    in1=t[:npart, :nseg, 64:],
                    op=mx,
                )
                nc.vector.tensor_tensor(
                    out=q[:npart, :nseg, :],
                    in0=h[:npart, :nseg, :32],
                    in1=h[:npart, :nseg, 32:],
                    op=mx,
                )
                nc.vector.tensor_tensor(
                    out=r3[:npart, :nseg, :],
                    in0=q[:npart, :nseg, :16],
                    in1=q[:npart, :nseg, 16:],
                    op=mx,
                )
                nc.vector.tensor_tensor(
                    out=r4[:npart, :nseg, :],
                    in0=r3[:npart, :nseg, :8],
                    in1=r3[:npart, :nseg, 8:],
                    op=mx,
                )
                return r4

            def tree_reduce(t, npart, nseg, kc_ap):
                """[npart, nseg, 128] u16 -> max over classes -> kc_ap [npart, nseg]."""
                r4 = _tree(t, npart, nseg)
                nc.vector.tensor_reduce(
                    out=kc_ap,
                    in_=r4[:npart, :nseg, :],
                    axis=mybir.AxisListType.X,
                    op=mx,
                )

            def tree_reduce2(t, kc_a, kc_b, w):
                """Full-S supertile whose columns straddle a chunk boundary."""
                r4 = _tree(t, 128, S)
                nc.vector.tensor_reduce(
                    out=kc_a,
                    in_=r4[:, :w, :],
                    axis=mybir.AxisListType.X,
                    op=mx,
                )
                nc.vector.tensor_reduce(
                    out=kc_b,
                    in_=r4[:, w:, :],
                    axis=mybir.AxisListType.X,
                    op=mx,
                )

            st_tiles = {}

            def load_st(si):
                t = inbuf.tile([128, S, C], u16, name="xt", tag="xt")
                eng = nc.sync if si % 2 == 0 else nc.scalar
                eng.dma_start(out=t[:], in_=xr[si])
                st_tiles[si] = t

            for si in range(1, min(6, NST)):
                load_st(si)

            # tail supertile rows 245760..249984 and mini-tail 249984..250000
            xt_tail = tailbuf.tile([128, TAIL_S, C], u16)
            nc.sync.dma_start(
                out=xt_tail[:],
                in_=x_ap[NST * ST_ROWS : NST * ST_ROWS + 128 * TAIL_S, :]
                .rearrange("(p k) c -> p (k c)", p=128, k=TAIL_S),
            )
            xt_tail2 = tailbuf.tile([TAIL2_P, 1, C], u16)
            nc.sync.dma_start(
                out=xt_tail2[:],
                in_=x_ap[NST * ST_ROWS + 128 * TAIL_S :, :].rearrange(
                    "(p k) c -> p (k c)", p=TAIL2_P, k=1
                ),
            )

            def chunk_of(j):
                c = 0
                while c + 1 < NCHUNKS and CHUNK_STARTS[c + 1] <= j:
                    c += 1
                return c

            fired = [0]
            for si in range(NST):
                if si == 0:
                    # supertile 0 from its 4 quarter tiles
                    for qi in range(4):
                        j = qi * QS
                        tree_reduce(
                            qtiles[qi], 128, QS, kc_tiles[0][:, j : j + QS]
                        )
                else:
                    t = st_tiles.pop(si)
                    if si + 5 < NST:
                        load_st(si + 5)
                    j = si * S
                    c = chunk_of(j)
                    l = j - CHUNK_STARTS[c]
                    if l + S <= CHUNK_SIZES[c]:
                        tree_reduce(t, 128, S, kc_tiles[c][:, l : l + S])
                    else:
                        # supertile straddles a chunk boundary: two TR halves
                        w = CHUNK_SIZES[c] - l
                        tree_reduce2(
                            t,
                            kc_tiles[c][:, l : l + w],
                            kc_tiles[c + 1][:, : S - w],
                            w,
                        )
                if si == 4:
                    # tail reduces mid-stream, away from busy start/end
                    tree_reduce(
                        xt_tail, 128, TAIL_S, kc_tiles[-1][:, :TAIL_S]
                    )
                    nc.vector.tensor_reduce(
                        out=kc_tiles[-1][:TAIL2_P, TAIL_S : TAIL_S + 1],
                        in_=xt_tail2[:],
                        axis=mybir.AxisListType.X,
                        op=mx,
                    )
                done_cols = (si + 1) * S
                while (
                    fired[0] < NCHUNKS - 2
                    and CHUNK_STARTS[fired[0]] + CHUNK_SIZES[fired[0]]
                    <= done_cols
                ):
                    emit_chunk_epilogue(fired[0])
                    fired[0] += 1
                if si == 6 and fired[0] >= 1:
                    # tail chunk fires early (its reduces ran at si == 4)
                    emit_chunk_epilogue(NCHUNKS - 1)

            while fired[0] < NCHUNKS - 1:
                emit_chunk_epilogue(fired[0], on_vector=fired[0] == NCHUNKS - 2)
                fired[0] += 1

            res = persist.tile([2 * MMB, 10 * MMB], f32)
            nc.vector.tensor_copy(out=res[:], in_=psum[:])
            nc.sync.dma_start(out=out[:], in_=res[:])

    nc.finalize()
    _NC_CACHE[key] = nc
    return nc


def _pack_keys(x_loc: np.ndarray, kscale: float) -> np.ndarray:
    """[ROWS, C] f32 -> uint16 keys q*128 + (127 - c)."""
    q = np.rint(x_loc * np.float32(kscale)).astype(np.uint16)
    inv_c = (127 - np.arange(C, dtype=np.uint16)).astype(np.uint16)
    return ((q << 7) | inv_c[None, :]).astype(np.uint16)


def _prep_targets(t_loc: np.ndarray) -> np.ndarray:
    """[ROWS] int targets -> [128, NTG] f32 holding 63.5 - target per column."""
    s0 = np.float32(63.5) - t_loc.astype(np.float32)
    tgc = np.full((128, NTG), -1000.5, dtype=np.float32)
    main = s0[: NST * ST_ROWS].reshape(NST, 128, S)
    tgc[:, : NST * S] = main.transpose(1, 0, 2).reshape(128, NST * S)
    tail = s0[NST * ST_ROWS : NST * ST_ROWS + 128 * TAIL_S].reshape(128, TAIL_S)
    tgc[:, NST * S : NST * S + TAIL_S] = tail
    tgc[:TAIL2_P, NTG - 1] = s0[NST * ST_ROWS + 128 * TAIL_S :]
    return tgc


def _run(inputs: np.ndarray, targets: np.ndarray, trace: bool = False):
    from concourse.bass_utils import run_bass_kernel_spmd

    inputs = np.ascontiguousarray(inputs, dtype=np.float32)
    targets = np.asarray(targets)
    vmax = float(inputs.max())
    kscale = _kscale(vmax)

    nc = _build_bass(kscale)

    in_maps = []
    for k in range(NCORES):
        lo = k * ROWS
        xk = _pack_keys(inputs[lo : lo + ROWS], kscale)
        tgc = _prep_targets(targets[lo : lo + ROWS])
        in_maps.append({"x": xk, "tg": tgc})

    last_err = None
    for _attempt in range(3):
        try:
            r = run_bass_kernel_spmd(
                nc, in_maps, core_ids=list(range(NCORES)), trace=trace
            )
            break
        except Exception as e:  # transient NRT_EXEC_UNIT_UNRECOVERABLE on cold device
            last_err = e
    else:
        raise last_err
    return r, kscale


def _combine(results, kscale) -> np.ndarray:
    Ssign = np.zeros((2, 10), dtype=np.float64)
    for r in results:
        o = r["out"].astype(np.float64).reshape(MMB, 2, MMB, 10)
        Ssign += np.einsum("aman->mn", o)
    # G_b in {-1,+1} for b>=1, G_0 = 1: [x >= E_b] = (S_b + S_0) / 2
    Ssign[:, 1:] = (Ssign[:, 1:] + Ssign[:, 0:1]) / 2.0
    Sq = Ssign[0] / (128.0 * kscale)   # cumulative conf sums (conf units)
    Sk = Ssign[1]                      # cumulative correct counts
    conf_sum = Sq - np.append(Sq[1:], 0.0)
    corr_sum = Sk - np.append(Sk[1:], 0.0)
    ece = np.abs(corr_sum - conf_sum).sum() / N
    return np.asarray(ece, dtype=np.float32)


def kernel(inputs: np.ndarray, targets: np.ndarray) -> np.ndarray:
    r, kscale = _run(inputs, targets, trace=False)
    return _combine(r.results, kscale)
